# revision 17
# baseline (speedup 1.0000x reference)
"""Cross-attention Trainium2 kernel (nn_CrossAttention, B=2, L=2048, D=1024,
Dctx=768, 16 heads x 64).

Sharding: 8 cores = 2 (batch) x 4 (head-groups of 4 heads). Each core computes
its batch's Q/K/V projections for its 4 heads, flash-style attention in the
transposed (S^T) domain, and a partial output projection; the host sums the
head-group partials and adds b_o.

All activations live transposed on-chip (xT, ctxT, qT, kT, attnT) so every
matmul contracts over the partition dim with no on-chip transposes. The host
ships x/context pre-transposed in fp16; all matmuls run in fp16 (full PE
streaming rate, half the SBUF/DMA traffic of fp32r, 2 more mantissa bits than
bf16). The kernel is software-pipelined around the Scalar engine's softmax
exp, which is the binding resource: attnV matmuls are emitted two key-tiles
behind the scores so the PE never waits on exp, and all projection / output
matmul chains are spread as filler work at explicit slots inside the
attention loops to keep the tensor engine continuously busy (which also
holds its fast p-state). The softmax denominator comes from 64 ones-rows
appended per head to V (written once by a memset); the d-block is copied to
SBUF, inverted with the fast-reciprocal DVE op, and applied to the PSUM
attention tile. The final output projection writes [128,1024] tiles with the
two PSUM->SBUF copies split across the Scalar and Vector engines.
"""
import numpy as np

import concourse.bass as bass
import concourse.tile as tile
from concourse import bacc, mybir, bass_utils

FP16 = mybir.dt.float16
F32 = mybir.dt.float32
EXP = mybir.ActivationFunctionType.Exp
CPY = mybir.ActivationFunctionType.Copy

# Problem shape (hardcoded per harness contract)
B, LQ, D = 2, 2048, 1024
DCTX = 768
NH, HD = 16, 64
SCALE = 1.0 / 8.0  # 1/sqrt(64)

# Per-core shard: 4 heads (one group), one batch
GH = 4                # heads per core
ONES = 64             # d-replication rows per head (memset, not matmul)
VW = HD + ONES        # 128: per-head width in augmented V
VAW = GH * VW         # 512
KT_Q = D // 128       # 8
KT_C = DCTX // 128    # 6
NLK = LQ // 128       # 16 key tiles
NSB = LQ // 1024      # 2 query 1024-slices (DMA granularity)
HALF = 1024
LAG = 2               # attnV trails scores by this many key tiles


def _build():
    nc = bacc.Bacc("TRN2", target_bir_lowering=False, debug=False,
                   enable_asserts=False, num_devices=8)

    xT_d = nc.dram_tensor("xT", (D, LQ), FP16, kind="ExternalInput").ap()
    cT_d = nc.dram_tensor("ctxT", (DCTX, LQ), FP16, kind="ExternalInput").ap()
    wq_d = nc.dram_tensor("wq", (D, 256), FP16, kind="ExternalInput").ap()
    wk_d = nc.dram_tensor("wk", (DCTX, 256), FP16, kind="ExternalInput").ap()
    wv_d = nc.dram_tensor("wv", (DCTX, 256), FP16, kind="ExternalInput").ap()
    wo_d = nc.dram_tensor("wo", (256, D), FP16, kind="ExternalInput").ap()
    bq_d = nc.dram_tensor("bq", (128, 2), F32, kind="ExternalInput").ap()
    bk_d = nc.dram_tensor("bk", (128, 2), F32, kind="ExternalInput").ap()
    bvb_d = nc.dram_tensor("bvb", (128, 256), F32, kind="ExternalInput").ap()
    out_d = nc.dram_tensor("outT", (D, LQ), F32, kind="ExternalOutput").ap()
    import os
    dbg = os.environ.get("KDBG") == "1"
    if dbg:
        dq_d = nc.dram_tensor("dbg_q", (128, 2 * LQ), FP16, kind="ExternalOutput").ap()
        dk_d = nc.dram_tensor("dbg_k", (128, 2 * LQ), FP16, kind="ExternalOutput").ap()
        dv_d = nc.dram_tensor("dbg_v", (128, NLK * VAW), FP16, kind="ExternalOutput").ap()
        da_d = nc.dram_tensor("dbg_a", (128, 2 * LQ), FP16, kind="ExternalOutput").ap()
        drd_d = nc.dram_tensor("dbg_rd", (ONES, HALF), F32, kind="ExternalOutput").ap()

    with tile.TileContext(nc) as tc:
        with tc.tile_pool(name="w", bufs=1) as wp, \
             tc.tile_pool(name="xt", bufs=16) as xtp, \
             tc.tile_pool(name="ct", bufs=12) as ctp, \
             tc.tile_pool(name="act", bufs=1) as actp, \
             tc.tile_pool(name="expp", bufs=5) as expp, \
             tc.tile_pool(name="rdp", bufs=2) as rdp, \
             tc.tile_pool(name="outp", bufs=3) as outp, \
             tc.tile_pool(name="ps_mm", bufs=2, space="PSUM") as ps_mm, \
             tc.tile_pool(name="ps_s", bufs=2, space="PSUM") as ps_s, \
             tc.tile_pool(name="ps_at", bufs=1, space="PSUM") as ps_at:

            # ---- DMAs in first-use order ----
            wk_t = wp.tile([128, KT_C * 256], FP16, tag="wk")
            nc.sync.dma_start(wk_t[:].rearrange("p (kt m) -> p kt m", m=256),
                              wk_d.rearrange("(kt p) m -> p kt m", p=128))
            bk_t = wp.tile([128, 2], F32, tag="bk")
            nc.sync.dma_start(bk_t[:], bk_d[:])
            ct_tiles = {}
            for kt in range(KT_C):
                t = ctp.tile([128, 1024], FP16, tag="ct")
                nc.sync.dma_start(t[:], cT_d[128 * kt:128 * (kt + 1), 0:1024])
                ct_tiles[(kt, 0)] = t
            wv_t = wp.tile([128, KT_C * 256], FP16, tag="wv")
            nc.sync.dma_start(wv_t[:].rearrange("p (kt m) -> p kt m", m=256),
                              wv_d.rearrange("(kt p) m -> p kt m", p=128))
            bvb_t = wp.tile([128, 256], F32, tag="bvb")
            nc.sync.dma_start(bvb_t[:], bvb_d[:])
            wq_t = wp.tile([128, KT_Q * 256], FP16, tag="wq")
            nc.sync.dma_start(wq_t[:].rearrange("p (kt m) -> p kt m", m=256),
                              wq_d.rearrange("(kt p) m -> p kt m", p=128))
            bq_t = wp.tile([128, 2], F32, tag="bq")
            nc.sync.dma_start(bq_t[:], bq_d[:])
            xt_tiles = {}
            for kt in range(KT_Q):
                t = xtp.tile([128, 1024], FP16, tag="xt")
                nc.sync.dma_start(t[:], xT_d[128 * kt:128 * (kt + 1), 0:1024])
                xt_tiles[(kt, 0)] = t
            for kt in range(KT_C):
                t = ctp.tile([128, 1024], FP16, tag="ct")
                nc.sync.dma_start(t[:], cT_d[128 * kt:128 * (kt + 1), 1024:2048])
                ct_tiles[(kt, 1)] = t
            for kt in range(KT_Q):
                t = xtp.tile([128, 1024], FP16, tag="xt")
                nc.sync.dma_start(t[:], xT_d[128 * kt:128 * (kt + 1), 1024:2048])
                xt_tiles[(kt, 1)] = t
            wo_t = wp.tile([128, 2 * D], FP16, tag="wo")
            nc.sync.dma_start(wo_t[:].rearrange("p (p2 m) -> p p2 m", m=1024),
                              wo_d.rearrange("(p2 p) m -> p p2 m", p=128))

            # ---- persistent activation tiles ----
            qT = [actp.tile([128, LQ], FP16, tag=f"qT{p}", name=f"qT{p}")
                  for p in range(2)]
            kT = [actp.tile([128, LQ], FP16, tag=f"kT{p}", name=f"kT{p}")
                  for p in range(2)]
            v_t = actp.tile([128, NLK * VAW], FP16, tag="v")
            aT = [actp.tile([128, LQ], FP16, tag=f"aT{p}", name=f"aT{p}")
                  for p in range(2)]

            # d-block ones: rows HD..VW of every per-head slot, written once
            ones_view = v_t[:].rearrange("p (j h w) -> p j h w",
                                         h=GH, w=VW)[:, :, :, HD:VW]
            nc.vector.memset(ones_view, 1.0)

            # ---- chain emitters (each is one PSUM accumulation) ----
            def k_chain(p, sb, n):
                ps = ps_mm.tile([128, 512], F32, tag="mm")
                for kt in range(KT_C):
                    nc.tensor.matmul(
                        ps[:], wk_t[:, 256 * kt + 128 * p:256 * kt + 128 * (p + 1)],
                        ct_tiles[(kt, sb)][:, 512 * n:512 * (n + 1)],
                        start=(kt == 0), stop=(kt == KT_C - 1))
                nc.vector.tensor_scalar_add(
                    kT[p][:, 1024 * sb + 512 * n:1024 * sb + 512 * (n + 1)],
                    ps[:], bk_t[:, p:p + 1])

            def q_chain(p, sb, n):
                ps = ps_mm.tile([128, 512], F32, tag="mm")
                for kt in range(KT_Q):
                    nc.tensor.matmul(
                        ps[:], wq_t[:, 256 * kt + 128 * p:256 * kt + 128 * (p + 1)],
                        xt_tiles[(kt, sb)][:, 512 * n:512 * (n + 1)],
                        start=(kt == 0), stop=(kt == KT_Q - 1))
                nc.vector.tensor_scalar_add(
                    qT[p][:, 1024 * sb + 512 * n:1024 * sb + 512 * (n + 1)],
                    ps[:], bq_t[:, p:p + 1])

            def v_chunk(j):
                # V rows for key chunk j: [128 ctx positions, 4 heads x 64]
                sb, jj = j // 8, j % 8
                ps = ps_mm.tile([128, 512], F32, tag="mm")
                for kt in range(KT_C):
                    nc.tensor.matmul(
                        ps[:, 0:256],
                        ct_tiles[(kt, sb)][:, 128 * jj:128 * (jj + 1)],
                        wv_t[:, 256 * kt:256 * (kt + 1)],
                        start=(kt == 0), stop=(kt == KT_C - 1))
                dst = v_t[:, VAW * j:VAW * (j + 1)].rearrange(
                    "p (h w) -> p h w", w=VW)[:, :, 0:HD]
                nc.vector.tensor_add(
                    dst, ps[:, 0:256].rearrange("p (h w) -> p h w", w=HD),
                    bvb_t[:].rearrange("p (h w) -> p h w", w=HD))

            def out_mm(s, mo):
                ps = ps_mm.tile([128, 512], F32, tag="mm")
                for p in range(2):
                    nc.tensor.matmul(
                        ps[:], wo_t[:, D * p + 128 * mo:D * p + 128 * (mo + 1)],
                        aT[p][:, 512 * s:512 * (s + 1)],
                        start=(p == 0), stop=(p == 1))
                return ps

            def out_unit(s, mo):
                # one [128,512] output slice: matmul + DVE copy + DMA
                ps = out_mm(s, mo)
                ot = outp.tile([128, 512], F32, tag="out")
                nc.vector.tensor_copy(ot[:], ps[:])
                nc.sync.dma_start(
                    out_d[128 * mo:128 * (mo + 1), 512 * s:512 * (s + 1)],
                    ot[:])

            def out_tail(mo):
                # [128,1024] tile for query cols 1024:2048; the two PSUM
                # copies run on Scalar and DVE in parallel, single DMA
                ot = outp.tile([128, 1024], F32, tag="outw")
                ps2 = out_mm(2, mo)
                nc.scalar.activation(ot[:, 0:512], ps2[:], CPY)
                ps3 = out_mm(3, mo)
                nc.vector.tensor_copy(ot[:, 512:1024], ps3[:])
                nc.sync.dma_start(
                    out_d[128 * mo:128 * (mo + 1), 1024:2048], ot[:])

            # ---- split projection chains: <=4-matmul filler segments ----
            # A segment pair (a, b) shares one open PSUM accumulation; the
            # schedule never overlaps two open pairs beyond ps_mm's 2 bufs.
            def k_seg(p, sb, n, box, lo, hi):
                if lo == 0:
                    box["ps"] = ps_mm.tile([128, 512], F32, tag="mm",
                                           name=f"kseg{p}{sb}{n}")
                ps = box["ps"]
                for kt in range(lo, hi):
                    nc.tensor.matmul(
                        ps[:], wk_t[:, 256 * kt + 128 * p:256 * kt + 128 * (p + 1)],
                        ct_tiles[(kt, sb)][:, 512 * n:512 * (n + 1)],
                        start=(kt == 0), stop=(kt == KT_C - 1))
                if hi == KT_C:
                    nc.vector.tensor_scalar_add(
                        kT[p][:, 1024 * sb + 512 * n:1024 * sb + 512 * (n + 1)],
                        ps[:], bk_t[:, p:p + 1])

            def q_seg(p, sb, n, box, lo, hi):
                if lo == 0:
                    box["ps"] = ps_mm.tile([128, 512], F32, tag="mm",
                                           name=f"qseg{p}{sb}{n}")
                ps = box["ps"]
                for kt in range(lo, hi):
                    nc.tensor.matmul(
                        ps[:], wq_t[:, 256 * kt + 128 * p:256 * kt + 128 * (p + 1)],
                        xt_tiles[(kt, sb)][:, 512 * n:512 * (n + 1)],
                        start=(kt == 0), stop=(kt == KT_Q - 1))
                if hi == KT_Q:
                    nc.vector.tensor_scalar_add(
                        qT[p][:, 1024 * sb + 512 * n:1024 * sb + 512 * (n + 1)],
                        ps[:], bq_t[:, p:p + 1])

            def k_pair(p, sb, n, t0):
                box = {}
                return {t0: lambda: k_seg(p, sb, n, box, 0, 3),
                        t0 + 2: lambda: k_seg(p, sb, n, box, 3, 6)}

            def q_pair(p, sb, n, t0):
                box = {}
                return {t0: lambda: q_seg(p, sb, n, box, 0, 4),
                        t0 + 2: lambda: q_seg(p, sb, n, box, 4, 8)}

            # ---- absolute filler slots over the 128-step stream ----
            # Constraints: kT[1]/qT[1] sb0 before t=32; kT[1] sb1 before
            # t=40; qT[0] sb1 before t=64; qT[1] sb1 before t=96; out s0/s1
            # after all half-0 normalizes (t>=64).
            slots = {}
            for d_ in (k_pair(1, 0, 0, 17), k_pair(1, 0, 1, 21),
                       q_pair(1, 0, 0, 25), q_pair(1, 0, 1, 29),
                       k_pair(1, 1, 0, 33), k_pair(1, 1, 1, 37),
                       q_pair(0, 1, 0, 41), q_pair(0, 1, 1, 45),
                       q_pair(1, 1, 0, 49), q_pair(1, 1, 1, 53)):
                slots.update(d_)
            for i in range(4):
                slots[65 + 4 * i] = lambda mo=i: out_unit(0, mo)
                slots[81 + 4 * i] = lambda mo=i + 4: out_unit(0, mo)
                slots[97 + 4 * i] = lambda mo=i: out_unit(1, mo)
                slots[113 + 4 * i] = lambda mo=i + 4: out_unit(1, mo)

            def attn_v(pa, h, j, ex):
                for n in range(2):
                    nc.tensor.matmul(
                        pa[:, 512 * n:512 * (n + 1)],
                        v_t[:, VAW * j + VW * h:VAW * j + VW * (h + 1)],
                        ex[:, 512 * n:512 * (n + 1)],
                        start=(j == 0), stop=(j == NLK - 1))

            def normalize(half, h, pa):
                # attnT = attnU * (1/d); d replicated in pa[64:128]
                p, m = h // 2, h % 2
                r0 = 64 * m
                dsb = rdp.tile([ONES, HALF], F32, tag="dsb")
                nc.vector.tensor_copy(dsb[:], pa[64:128, :])
                rd = rdp.tile([ONES, HALF], F32, tag="rd")
                nc.vector.reciprocal_approx_fast(rd[:], dsb[:])
                if dbg and h == 0 and half == 0:
                    nc.sync.dma_start(drd_d[:], rd[:])
                cols = slice(HALF * half, HALF * (half + 1))
                nc.vector.tensor_mul(aT[p][r0:r0 + 64, cols], pa[0:64, :], rd[:])

            # ---- prologue: everything stream step 0 needs ----
            for n in range(2):
                k_chain(0, 0, n)
            for n in range(2):
                k_chain(0, 1, n)
            v_chunk(0)
            v_chunk(1)
            for n in range(2):
                q_chain(0, 0, n)

            # ---- one continuous stream over all 8 attention blocks ----
            pa_tiles = {}
            ex_tiles = {}
            for t in range(16 * 2 * GH + LAG):
                b, i = divmod(t, NLK)
                if t < 16 * 2 * GH:
                    half, h = divmod(b, GH)
                    p, m = h // 2, h % 2
                    r0 = 64 * m
                    if b == 0 and i >= 2:
                        v_chunk(i)
                    st = ps_s.tile([128, HALF], F32, tag="s")
                    for n in range(2):
                        nc.tensor.matmul(
                            st[:, 512 * n:512 * (n + 1)],
                            kT[p][r0:r0 + 64, 128 * i:128 * (i + 1)],
                            qT[p][r0:r0 + 64,
                                  HALF * half + 512 * n:HALF * half + 512 * (n + 1)],
                            start=True, stop=True)
                    ex = expp.tile([128, HALF], FP16, tag="expS")
                    nc.scalar.activation(ex[:], st[:], EXP, scale=SCALE)
                    ex_tiles[t] = ex
                tp = t - LAG
                if tp >= 0:
                    bp, ip = divmod(tp, NLK)
                    halfp, hp = divmod(bp, GH)
                    if ip == 0:
                        pa_tiles[bp] = ps_at.tile([128, HALF], F32, tag="attn",
                                                  name=f"pa{bp}")
                    attn_v(pa_tiles[bp], hp, ip, ex_tiles.pop(tp))
                    if ip == NLK - 1:
                        normalize(halfp, hp, pa_tiles.pop(bp))
                if t in slots:
                    slots[t]()

            for mo in range(D // 128):
                out_tail(mo)

            if dbg:
                for p in range(2):
                    nc.sync.dma_start(dq_d[:, LQ * p:LQ * (p + 1)], qT[p][:])
                    nc.sync.dma_start(dk_d[:, LQ * p:LQ * (p + 1)], kT[p][:])
                    nc.sync.dma_start(da_d[:, LQ * p:LQ * (p + 1)], aT[p][:])
                nc.sync.dma_start(dv_d[:], v_t[:])

    nc.compile()
    return nc


_NC_CACHE = []


def _get_nc():
    if not _NC_CACHE:
        _NC_CACHE.append(_build())
    return _NC_CACHE[0]


def kernel_run(inputs, trace=False, **kw):
    """Run on HW; returns (full_output, BassKernelResults)."""
    x = np.asarray(inputs["x"], np.float32)
    context = np.asarray(inputs["context"], np.float32)
    w_q = np.asarray(inputs["w_q"], np.float32)
    b_q = np.asarray(inputs["b_q"], np.float32)
    w_k = np.asarray(inputs["w_k"], np.float32)
    b_k = np.asarray(inputs["b_k"], np.float32)
    w_v = np.asarray(inputs["w_v"], np.float32)
    b_v = np.asarray(inputs["b_v"], np.float32)
    w_o = np.asarray(inputs["w_o"], np.float32)
    b_o = np.asarray(inputs["b_o"], np.float32)

    f16 = np.float16
    xT = [np.ascontiguousarray(x[b].T).astype(f16) for b in range(B)]
    cT = [np.ascontiguousarray(context[b].T).astype(f16) for b in range(B)]

    maps = []
    for c in range(8):
        b, g = c // 4, c % 4
        hs = slice(256 * g, 256 * (g + 1))
        maps.append({
            "xT": xT[b],
            "ctxT": cT[b],
            "wq": np.ascontiguousarray(w_q[:, hs]).astype(f16),
            "wk": np.ascontiguousarray(w_k[:, hs]).astype(f16),
            "wv": np.ascontiguousarray(w_v[:, hs]).astype(f16),
            "wo": np.ascontiguousarray(w_o[hs, :]).astype(f16),
            "bq": np.ascontiguousarray(b_q[hs].reshape(2, 128).T.astype(np.float32)),
            "bk": np.ascontiguousarray(b_k[hs].reshape(2, 128).T.astype(np.float32)),
            "bvb": np.broadcast_to(b_v[hs].astype(np.float32), (128, 256)).copy(),
        })

    nc = _get_nc()
    res = bass_utils.run_bass_kernel_spmd(nc, maps, core_ids=list(range(8)),
                                          trace=trace, **kw)
    out = np.empty((B, LQ, D), np.float32)
    for b in range(B):
        acc = res.results[4 * b]["outT"].astype(np.float32)
        for g in range(1, 4):
            acc = acc + res.results[4 * b + g]["outT"]
        out[b] = acc.T + b_o[None, :]
    return out, res


def kernel(**inputs) -> np.ndarray:
    out, _ = kernel_run(inputs)
    return out


# revision 24
# speedup vs baseline: 1.1116x; 1.1116x over previous
"""Cross-attention Trainium2 kernel (nn_CrossAttention, B=2, L=2048, D=1024,
Dctx=768, 16 heads x 64).

Sharding: 8 cores = 2 (batch) x 4 (head-groups of 4 heads). Each core computes
its batch's Q/K/V projections for its 4 heads, flash-style attention in the
transposed (S^T) domain, and a partial output projection; the host sums the
head-group partials and adds b_o.

All activations live transposed on-chip (xT, ctxT, qT, kT, attnT) so every
matmul contracts over the partition dim with no on-chip transposes. The host
ships x/context pre-transposed in fp16; all matmuls run in fp16 (full PE
streaming rate, half the SBUF/DMA traffic of fp32r, 2 more mantissa bits than
bf16). The kernel is software-pipelined around the Scalar engine's softmax
exp, which is the binding resource: attnV matmuls are emitted two key-tiles
behind the scores so the PE never waits on exp, and all projection / output
matmul chains are spread as filler work at explicit slots inside the
attention loops to keep the tensor engine continuously busy (which also
holds its fast p-state). The softmax denominator comes from 64 ones-rows
appended per head to V (written once by a memset); the d-block is copied to
SBUF, inverted with the fast-reciprocal DVE op, and applied to the PSUM
attention tile. The final output projection writes [128,1024] tiles with the
two PSUM->SBUF copies split across the Scalar and Vector engines.
"""
import numpy as np

import concourse.bass as bass
import concourse.tile as tile
from concourse import bacc, mybir, bass_utils

FP16 = mybir.dt.float16
F32 = mybir.dt.float32
EXP = mybir.ActivationFunctionType.Exp
CPY = mybir.ActivationFunctionType.Copy

# Problem shape (hardcoded per harness contract)
B, LQ, D = 2, 2048, 1024
DCTX = 768
NH, HD = 16, 64
SCALE = 1.0 / 8.0  # 1/sqrt(64)

# Per-core shard: 4 heads (one group), one batch
GH = 4                # heads per core
ONES = 64             # d-replication rows per head (memset, not matmul)
VW = HD + ONES        # 128: per-head width in augmented V
VAW = GH * VW         # 512
KT_Q = D // 128       # 8
KT_C = DCTX // 128    # 6
NLK = LQ // 128       # 16 key tiles
NSB = LQ // 1024      # 2 query 1024-slices (DMA granularity)
HALF = 1024
LAG = 2               # attnV trails scores by this many key tiles


def _build():
    nc = bacc.Bacc("TRN2", target_bir_lowering=False, debug=False,
                   enable_asserts=False, num_devices=8)

    xT_d = nc.dram_tensor("xT", (D, LQ), FP16, kind="ExternalInput").ap()
    cT_d = nc.dram_tensor("ctxT", (DCTX, LQ), FP16, kind="ExternalInput").ap()
    wq_d = nc.dram_tensor("wq", (D, 256), FP16, kind="ExternalInput").ap()
    wk_d = nc.dram_tensor("wk", (DCTX, 256), FP16, kind="ExternalInput").ap()
    wv_d = nc.dram_tensor("wv", (DCTX, 256), FP16, kind="ExternalInput").ap()
    wo_d = nc.dram_tensor("wo", (256, D), FP16, kind="ExternalInput").ap()
    bq_d = nc.dram_tensor("bq", (128, 2), F32, kind="ExternalInput").ap()
    bk_d = nc.dram_tensor("bk", (128, 2), F32, kind="ExternalInput").ap()
    bvb_d = nc.dram_tensor("bvb", (128, 256), F32, kind="ExternalInput").ap()
    out_d = nc.dram_tensor("outT", (D, LQ), F32, kind="ExternalOutput").ap()
    import os
    dbg = os.environ.get("KDBG") == "1"
    if dbg:
        dq_d = nc.dram_tensor("dbg_q", (128, 2 * LQ), FP16, kind="ExternalOutput").ap()
        dk_d = nc.dram_tensor("dbg_k", (128, 2 * LQ), FP16, kind="ExternalOutput").ap()
        dv_d = nc.dram_tensor("dbg_v", (128, NLK * VAW), FP16, kind="ExternalOutput").ap()
        da_d = nc.dram_tensor("dbg_a", (128, 2 * LQ), FP16, kind="ExternalOutput").ap()
        drd_d = nc.dram_tensor("dbg_rd", (ONES, HALF), F32, kind="ExternalOutput").ap()

    with tile.TileContext(nc) as tc:
        with tc.tile_pool(name="w", bufs=1) as wp, \
             tc.tile_pool(name="xt", bufs=16) as xtp, \
             tc.tile_pool(name="ct", bufs=12) as ctp, \
             tc.tile_pool(name="act", bufs=1) as actp, \
             tc.tile_pool(name="expp", bufs=5) as expp, \
             tc.tile_pool(name="rdp", bufs=2) as rdp, \
             tc.tile_pool(name="outp", bufs=3) as outp, \
             tc.tile_pool(name="ps_mm", bufs=2, space="PSUM") as ps_mm, \
             tc.tile_pool(name="ps_s", bufs=2, space="PSUM") as ps_s, \
             tc.tile_pool(name="ps_at", bufs=1, space="PSUM") as ps_at:

            # ---- DMAs in first-use order ----
            wk_t = wp.tile([128, KT_C * 256], FP16, tag="wk")
            nc.sync.dma_start(wk_t[:].rearrange("p (kt m) -> p kt m", m=256),
                              wk_d.rearrange("(kt p) m -> p kt m", p=128))
            bk_t = wp.tile([128, 2], F32, tag="bk")
            nc.sync.dma_start(bk_t[:], bk_d[:])
            ct_tiles = {}
            for kt in range(KT_C):
                t = ctp.tile([128, 1024], FP16, tag="ct")
                nc.sync.dma_start(t[:], cT_d[128 * kt:128 * (kt + 1), 0:1024])
                ct_tiles[(kt, 0)] = t
            wv_t = wp.tile([128, KT_C * 256], FP16, tag="wv")
            nc.sync.dma_start(wv_t[:].rearrange("p (kt m) -> p kt m", m=256),
                              wv_d.rearrange("(kt p) m -> p kt m", p=128))
            bvb_t = wp.tile([128, 256], F32, tag="bvb")
            nc.sync.dma_start(bvb_t[:], bvb_d[:])
            wq_t = wp.tile([128, KT_Q * 256], FP16, tag="wq")
            nc.sync.dma_start(wq_t[:].rearrange("p (kt m) -> p kt m", m=256),
                              wq_d.rearrange("(kt p) m -> p kt m", p=128))
            bq_t = wp.tile([128, 2], F32, tag="bq")
            nc.sync.dma_start(bq_t[:], bq_d[:])
            xt_tiles = {}
            for kt in range(KT_Q):
                t = xtp.tile([128, 1024], FP16, tag="xt")
                nc.sync.dma_start(t[:], xT_d[128 * kt:128 * (kt + 1), 0:1024])
                xt_tiles[(kt, 0)] = t
            for kt in range(KT_C):
                t = ctp.tile([128, 1024], FP16, tag="ct")
                nc.sync.dma_start(t[:], cT_d[128 * kt:128 * (kt + 1), 1024:2048])
                ct_tiles[(kt, 1)] = t
            for kt in range(KT_Q):
                t = xtp.tile([128, 1024], FP16, tag="xt")
                nc.sync.dma_start(t[:], xT_d[128 * kt:128 * (kt + 1), 1024:2048])
                xt_tiles[(kt, 1)] = t
            wo_t = wp.tile([128, 2 * D], FP16, tag="wo")
            nc.sync.dma_start(wo_t[:].rearrange("p (p2 m) -> p p2 m", m=1024),
                              wo_d.rearrange("(p2 p) m -> p p2 m", p=128))

            # ---- persistent activation tiles ----
            # qz[h]: per-head Q with the other head's 64 rows zeroed, so the
            # scores matmul can contract over the full 128 partitions (the
            # 64-row tile mode streams measurably slower on HW).
            qz = [actp.tile([128, LQ], FP16, tag=f"qz{h}", name=f"qz{h}")
                  for h in range(GH)]
            kT = [actp.tile([128, LQ], FP16, tag=f"kT{p}", name=f"kT{p}")
                  for p in range(2)]
            v_t = actp.tile([128, NLK * VAW], FP16, tag="v")
            aT = [actp.tile([128, LQ], FP16, tag=f"aT{p}", name=f"aT{p}")
                  for p in range(2)]

            # d-block ones: rows HD..VW of every per-head slot, written once
            ones_view = v_t[:].rearrange("p (j h w) -> p j h w",
                                         h=GH, w=VW)[:, :, :, HD:VW]
            nc.vector.memset(ones_view, 1.0)
            # zero the complementary head half of each qz tile (gpsimd: off
            # the critical path, SBUF-only)
            for h in range(GH):
                nc.gpsimd.memset(qz[h][(64 if h % 2 == 0 else 0):
                                       (128 if h % 2 == 0 else 64), :], 0.0)

            # ---- chain emitters (each is one PSUM accumulation) ----
            def k_chain(p, sb, n):
                ps = ps_mm.tile([128, 512], F32, tag="mm")
                for kt in range(KT_C):
                    nc.tensor.matmul(
                        ps[:], wk_t[:, 256 * kt + 128 * p:256 * kt + 128 * (p + 1)],
                        ct_tiles[(kt, sb)][:, 512 * n:512 * (n + 1)],
                        start=(kt == 0), stop=(kt == KT_C - 1))
                nc.vector.tensor_scalar_add(
                    kT[p][:, 1024 * sb + 512 * n:1024 * sb + 512 * (n + 1)],
                    ps[:], bk_t[:, p:p + 1])

            def q_bias_split(p, sb, n, ps):
                cols = slice(1024 * sb + 512 * n, 1024 * sb + 512 * (n + 1))
                nc.vector.tensor_scalar_add(
                    qz[2 * p][0:64, cols], ps[0:64, :], bq_t[0:64, p:p + 1])
                nc.vector.tensor_scalar_add(
                    qz[2 * p + 1][64:128, cols], ps[64:128, :],
                    bq_t[64:128, p:p + 1])

            def q_chain(p, sb, n):
                ps = ps_mm.tile([128, 512], F32, tag="mm")
                for kt in range(KT_Q):
                    nc.tensor.matmul(
                        ps[:], wq_t[:, 256 * kt + 128 * p:256 * kt + 128 * (p + 1)],
                        xt_tiles[(kt, sb)][:, 512 * n:512 * (n + 1)],
                        start=(kt == 0), stop=(kt == KT_Q - 1))
                q_bias_split(p, sb, n, ps)

            def v_chunk(j):
                # V rows for key chunk j: [128 ctx positions, 4 heads x 64]
                sb, jj = j // 8, j % 8
                ps = ps_mm.tile([128, 512], F32, tag="mm")
                for kt in range(KT_C):
                    nc.tensor.matmul(
                        ps[:, 0:256],
                        ct_tiles[(kt, sb)][:, 128 * jj:128 * (jj + 1)],
                        wv_t[:, 256 * kt:256 * (kt + 1)],
                        start=(kt == 0), stop=(kt == KT_C - 1))
                dst = v_t[:, VAW * j:VAW * (j + 1)].rearrange(
                    "p (h w) -> p h w", w=VW)[:, :, 0:HD]
                nc.vector.tensor_add(
                    dst, ps[:, 0:256].rearrange("p (h w) -> p h w", w=HD),
                    bvb_t[:].rearrange("p (h w) -> p h w", w=HD))

            def out_mm(s, mo):
                ps = ps_mm.tile([128, 512], F32, tag="mm")
                for p in range(2):
                    nc.tensor.matmul(
                        ps[:], wo_t[:, D * p + 128 * mo:D * p + 128 * (mo + 1)],
                        aT[p][:, 512 * s:512 * (s + 1)],
                        start=(p == 0), stop=(p == 1))
                return ps

            def out_unit(s, mo):
                # one [128,512] output slice: matmul + DVE copy + DMA
                ps = out_mm(s, mo)
                ot = outp.tile([128, 512], F32, tag="out")
                nc.vector.tensor_copy(ot[:], ps[:])
                nc.sync.dma_start(
                    out_d[128 * mo:128 * (mo + 1), 512 * s:512 * (s + 1)],
                    ot[:])

            def out_tail(mo):
                # [128,1024] tile for query cols 1024:2048; the two PSUM
                # copies run on Scalar and DVE in parallel, single DMA
                ot = outp.tile([128, 1024], F32, tag="outw")
                ps2 = out_mm(2, mo)
                nc.scalar.activation(ot[:, 0:512], ps2[:], CPY)
                ps3 = out_mm(3, mo)
                nc.vector.tensor_copy(ot[:, 512:1024], ps3[:])
                nc.sync.dma_start(
                    out_d[128 * mo:128 * (mo + 1), 1024:2048], ot[:])

            # ---- split projection chains: <=4-matmul filler segments ----
            # A segment pair (a, b) shares one open PSUM accumulation; the
            # schedule never overlaps two open pairs beyond ps_mm's 2 bufs.
            def k_seg(p, sb, n, box, lo, hi):
                if lo == 0:
                    box["ps"] = ps_mm.tile([128, 512], F32, tag="mm",
                                           name=f"kseg{p}{sb}{n}")
                ps = box["ps"]
                for kt in range(lo, hi):
                    nc.tensor.matmul(
                        ps[:], wk_t[:, 256 * kt + 128 * p:256 * kt + 128 * (p + 1)],
                        ct_tiles[(kt, sb)][:, 512 * n:512 * (n + 1)],
                        start=(kt == 0), stop=(kt == KT_C - 1))
                if hi == KT_C:
                    nc.vector.tensor_scalar_add(
                        kT[p][:, 1024 * sb + 512 * n:1024 * sb + 512 * (n + 1)],
                        ps[:], bk_t[:, p:p + 1])

            def q_seg(p, sb, n, box, lo, hi):
                if lo == 0:
                    box["ps"] = ps_mm.tile([128, 512], F32, tag="mm",
                                           name=f"qseg{p}{sb}{n}")
                ps = box["ps"]
                for kt in range(lo, hi):
                    nc.tensor.matmul(
                        ps[:], wq_t[:, 256 * kt + 128 * p:256 * kt + 128 * (p + 1)],
                        xt_tiles[(kt, sb)][:, 512 * n:512 * (n + 1)],
                        start=(kt == 0), stop=(kt == KT_Q - 1))
                if hi == KT_Q:
                    q_bias_split(p, sb, n, ps)

            def k_pair(p, sb, n, t0):
                box = {}
                return {t0: lambda: k_seg(p, sb, n, box, 0, 3),
                        t0 + 2: lambda: k_seg(p, sb, n, box, 3, 6)}

            def q_pair(p, sb, n, t0):
                box = {}
                return {t0: lambda: q_seg(p, sb, n, box, 0, 4),
                        t0 + 2: lambda: q_seg(p, sb, n, box, 4, 8)}

            # ---- absolute filler slots over the 128-step stream ----
            # Constraints: kT[1]/qT[1] sb0 before t=32; kT[1] sb1 before
            # t=40; qT[0] sb1 before t=64; qT[1] sb1 before t=96; out s0/s1
            # after all half-0 normalizes (t>=64).
            slots = {}
            for d_ in (k_pair(0, 1, 0, 3), k_pair(0, 1, 1, 7),
                       k_pair(1, 0, 0, 17), k_pair(1, 0, 1, 21),
                       q_pair(1, 0, 0, 25), q_pair(1, 0, 1, 29),
                       k_pair(1, 1, 0, 33), k_pair(1, 1, 1, 37),
                       q_pair(0, 1, 0, 41), q_pair(0, 1, 1, 45),
                       q_pair(1, 1, 0, 49), q_pair(1, 1, 1, 53)):
                slots.update(d_)
            for i in range(4):
                slots[65 + 4 * i] = lambda mo=i: out_unit(0, mo)
                slots[81 + 4 * i] = lambda mo=i + 4: out_unit(0, mo)
                slots[97 + 4 * i] = lambda mo=i: out_unit(1, mo)
                slots[113 + 4 * i] = lambda mo=i + 4: out_unit(1, mo)

            def attn_v(pa, h, j, ex):
                for n in range(2):
                    nc.tensor.matmul(
                        pa[:, 512 * n:512 * (n + 1)],
                        v_t[:, VAW * j + VW * h:VAW * j + VW * (h + 1)],
                        ex[:, 512 * n:512 * (n + 1)],
                        start=(j == 0), stop=(j == NLK - 1))

            def normalize(half, h, pa):
                # attnT = attnU * (1/d); d replicated in pa[64:128]
                p, m = h // 2, h % 2
                r0 = 64 * m
                dsb = rdp.tile([ONES, HALF], F32, tag="dsb")
                nc.vector.tensor_copy(dsb[:], pa[64:128, :])
                rd = rdp.tile([ONES, HALF], F32, tag="rd")
                nc.vector.reciprocal_approx_fast(rd[:], dsb[:])
                if dbg and h == 0 and half == 0:
                    nc.sync.dma_start(drd_d[:], rd[:])
                cols = slice(HALF * half, HALF * (half + 1))
                nc.vector.tensor_mul(aT[p][r0:r0 + 64, cols], pa[0:64, :], rd[:])

            # ---- prologue: everything stream step 0 needs ----
            for n in range(2):
                k_chain(0, 0, n)
            v_chunk(0)
            v_chunk(1)
            for n in range(2):
                q_chain(0, 0, n)

            # ---- one continuous stream over all 8 attention blocks ----
            pa_tiles = {}
            ex_tiles = {}
            for t in range(16 * 2 * GH + LAG):
                b, i = divmod(t, NLK)
                if t < 16 * 2 * GH:
                    half, h = divmod(b, GH)
                    p, m = h // 2, h % 2
                    r0 = 64 * m
                    if b == 0 and i >= 2:
                        v_chunk(i)
                    st = ps_s.tile([128, HALF], F32, tag="s")
                    for n in range(2):
                        nc.tensor.matmul(
                            st[:, 512 * n:512 * (n + 1)],
                            kT[p][:, 128 * i:128 * (i + 1)],
                            qz[h][:, HALF * half + 512 * n:
                                  HALF * half + 512 * (n + 1)],
                            start=True, stop=True)
                    ex = expp.tile([128, HALF], FP16, tag="expS")
                    nc.scalar.activation(ex[:], st[:], EXP, scale=SCALE)
                    ex_tiles[t] = ex
                tp = t - LAG
                if tp >= 0:
                    bp, ip = divmod(tp, NLK)
                    halfp, hp = divmod(bp, GH)
                    if ip == 0:
                        pa_tiles[bp] = ps_at.tile([128, HALF], F32, tag="attn",
                                                  name=f"pa{bp}")
                    attn_v(pa_tiles[bp], hp, ip, ex_tiles.pop(tp))
                    if ip == NLK - 1:
                        normalize(halfp, hp, pa_tiles.pop(bp))
                if t in slots:
                    slots[t]()

            for mo in range(D // 128):
                out_tail(mo)

            if dbg:
                for p in range(2):
                    nc.sync.dma_start(dq_d[:, LQ * p:LQ * (p + 1)],
                                      qz[3 * p][:])
                    nc.sync.dma_start(dk_d[:, LQ * p:LQ * (p + 1)], kT[p][:])
                    nc.sync.dma_start(da_d[:, LQ * p:LQ * (p + 1)], aT[p][:])
                nc.sync.dma_start(dv_d[:], v_t[:])

    nc.compile()
    return nc


_NC_CACHE = []


def _get_nc():
    if not _NC_CACHE:
        _NC_CACHE.append(_build())
    return _NC_CACHE[0]


def kernel_run(inputs, trace=False, **kw):
    """Run on HW; returns (full_output, BassKernelResults)."""
    x = np.asarray(inputs["x"], np.float32)
    context = np.asarray(inputs["context"], np.float32)
    w_q = np.asarray(inputs["w_q"], np.float32)
    b_q = np.asarray(inputs["b_q"], np.float32)
    w_k = np.asarray(inputs["w_k"], np.float32)
    b_k = np.asarray(inputs["b_k"], np.float32)
    w_v = np.asarray(inputs["w_v"], np.float32)
    b_v = np.asarray(inputs["b_v"], np.float32)
    w_o = np.asarray(inputs["w_o"], np.float32)
    b_o = np.asarray(inputs["b_o"], np.float32)

    f16 = np.float16
    xT = [np.ascontiguousarray(x[b].T).astype(f16) for b in range(B)]
    cT = [np.ascontiguousarray(context[b].T).astype(f16) for b in range(B)]

    maps = []
    for c in range(8):
        b, g = c // 4, c % 4
        hs = slice(256 * g, 256 * (g + 1))
        maps.append({
            "xT": xT[b],
            "ctxT": cT[b],
            "wq": np.ascontiguousarray(w_q[:, hs]).astype(f16),
            "wk": np.ascontiguousarray(w_k[:, hs]).astype(f16),
            "wv": np.ascontiguousarray(w_v[:, hs]).astype(f16),
            "wo": np.ascontiguousarray(w_o[hs, :]).astype(f16),
            "bq": np.ascontiguousarray(b_q[hs].reshape(2, 128).T.astype(np.float32)),
            "bk": np.ascontiguousarray(b_k[hs].reshape(2, 128).T.astype(np.float32)),
            "bvb": np.broadcast_to(b_v[hs].astype(np.float32), (128, 256)).copy(),
        })

    nc = _get_nc()
    res = bass_utils.run_bass_kernel_spmd(nc, maps, core_ids=list(range(8)),
                                          trace=trace, **kw)
    out = np.empty((B, LQ, D), np.float32)
    for b in range(B):
        acc = res.results[4 * b]["outT"].astype(np.float32)
        for g in range(1, 4):
            acc = acc + res.results[4 * b + g]["outT"]
        out[b] = acc.T + b_o[None, :]
    return out, res


def kernel(**inputs) -> np.ndarray:
    out, _ = kernel_run(inputs)
    return out
